# revision 5
# baseline (speedup 1.0000x reference)
"""Trainium2 Bass kernel for AdditiveAttentionSACModel (v2).

Data-parallel over 8 NeuronCores, BC=4096 samples/core, 8 tiles of 512.
Feature-major on-chip layout: ATTN_D=128 on partitions, (intruder, sample)
tokens on the free dim.

Key structural choices (vs the v1 kernel this replaces):
  - Host-side compaction: samples are globally sorted by live-intruder
    count and padded slots are dropped; each tile processes only
    c_t = max live count in tile slots (~76% of the 32 slots on average).
    Pad residue inside a tile is masked with a -1e30 additive matmul.
  - Softmax without transposes or max-subtraction: scores stay [32, b],
    sum over slots via a ones-matmul (contracting only c rows), exp has no
    overflow risk (|scores| <= sum|v_att| ~ 14).
  - Alpha broadcast to 128 partitions via DRAM round-trip DMA (idle
    fabric), normalized alphas multiply ie on DVE in 2x bf16 mode;
    products reduce with a halving tree split across DVE and Pool.
  - Wv @ proj_W precombined on host: ctx/proj collapse into one matmul.
  - lrelu evacuations of z are load-balanced across Act (1 op Prelu),
    DVE (2 ops), and DVE-copy+Pool-stt routes.
  - y is produced feature-major [2, bc] and fixed up on host.
Weights/biases: own/int biases ride a constant-one feature row (exact for
any values); other biases are all zero in this model - the host checks and
falls back to extra bias matmuls if they are ever nonzero.
"""

import numpy as np
import ml_dtypes

import concourse.bass as bass
import concourse.bacc as bacc
import concourse.mybir as mybir
import concourse.tile as tile
from concourse.ap import AP
from contextlib import ExitStack

# ---- problem constants (hardcoded; kernel.py must be self-contained) ----
N_CORES = 8
B_FULL = 32768
BC = B_FULL // N_CORES          # 4096 samples per core
NI = 32
OWN_D = 3
INT_D = 7
D = 128
HID = 256
OUT_D = 2
OBS_D = OWN_D + NI * INT_D      # 227
NEG_SLOPE = 0.2

B_TILE = 512
NT = BC // B_TILE               # 8 tiles per core
NEG_BIG = -1.0e30

F32 = mybir.dt.float32
BF16 = mybir.dt.bfloat16
AF = mybir.ActivationFunctionType
ALU = mybir.AluOpType
BF16_NP = ml_dtypes.bfloat16

# ---- tuning knobs ----
# lrelu evac route per slot index: 'A' Act Prelu, 'D' DVE 2-op,
# 'P' DVE copy + Pool scalar_tensor_tensor
LRELU_PATTERN = "AADAD"
Q_DVE_EVERY = 0      # every k-th q-add on DVE instead of PE (0 = none)
TREE_POOL_FRAC = 0.5   # column fraction of each tree level done on Pool
MUL_POOL_EVERY = 3   # every k-th alpha-mul op on Pool (0 = none)
EXB_GROUP = 8        # slots per alpha-broadcast DMA


def _lrelu_route(n):
    return LRELU_PATTERN[n % len(LRELU_PATTERN)]


def build_program(cs, bc=BC, b_tile=B_TILE, with_bias_mms=False):
    """Per-core Bass program. cs = per-tile live-slot counts (same on all
    cores; SPMD). with_bias_mms adds explicit bias matmuls for the head
    (needed only if proj/h1/h2/out biases are nonzero)."""
    nt = len(cs)
    assert nt * b_tile == bc
    cmax = max(cs)
    tot_cols = sum(c * b_tile for c in cs)

    nc = bacc.Bacc("TRN2", target_bir_lowering=False, debug=False,
                   num_devices=N_CORES)

    def din(name, shape, dt=BF16):
        return nc.dram_tensor(name, list(shape), dt, kind="ExternalInput")

    intrT = din("intrT", [INT_D + 1, tot_cols])    # compacted tokens
    ownT = din("ownT", [OWN_D + 1, bc])
    maskT = din("maskT", [nt, NI, b_tile])          # -1e30 at pad slots
    ownW = din("ownW", [OWN_D + 1, D])
    intW = din("intW", [INT_D + 1, D])
    wq = din("wq", [D, D])
    wk = din("wk", [D, D])
    vattm = din("vattm", [D, NI * NI])
    id32 = din("id32", [NI, NI])
    ones32 = din("ones32", [NI, 1])
    wvp = din("wvp", [D, D])
    h1oe0 = din("h1oe0", [D, D]); h1oe1 = din("h1oe1", [D, D])
    h1at0 = din("h1at0", [D, D]); h1at1 = din("h1at1", [D, D])
    h2a0 = din("h2a0", [D, D]); h2a1 = din("h2a1", [D, D])
    h2b0 = din("h2b0", [D, D]); h2b1 = din("h2b1", [D, D])
    outlo = din("outlo", [D, OUT_D]); outhi = din("outhi", [D, OUT_D])
    biasrows = din("biasrows", [1, 6 * D])   # projb,h1b0,h1b1,h2b0,h2b1,outb
    onesrow = din("onesrow", [1, b_tile])

    exd = nc.dram_tensor("exd", [nt, NI * b_tile], BF16, kind="Internal")
    y = nc.dram_tensor("y", [OUT_D, bc], F32, kind="ExternalOutput")

    with tile.TileContext(nc) as tc, ExitStack() as ctx:
        # PSUM: pz 2 + pe 4 + sct 1 + pm 1 = 8 banks
        p_z = ctx.enter_context(tc.tile_pool(name="p_z", bufs=2, space="PSUM"))
        p_e = ctx.enter_context(tc.tile_pool(name="p_e", bufs=2, space="PSUM"))
        p_s = ctx.enter_context(tc.tile_pool(name="p_s", bufs=1, space="PSUM"))
        p_m = ctx.enter_context(tc.tile_pool(name="p_m", bufs=1, space="PSUM"))

        wp = ctx.enter_context(tc.tile_pool(name="wp", bufs=1))
        s_intr = ctx.enter_context(tc.tile_pool(name="s_intr", bufs=2))
        s_mask = ctx.enter_context(tc.tile_pool(name="s_mask", bufs=2))
        s_ie = ctx.enter_context(tc.tile_pool(name="s_ie", bufs=2))
        s_oe = ctx.enter_context(tc.tile_pool(name="s_oe", bufs=3))
        s_ech = ctx.enter_context(tc.tile_pool(name="s_ech", bufs=2))
        s_zs = ctx.enter_context(tc.tile_pool(name="s_zs", bufs=3))
        s_ex = ctx.enter_context(tc.tile_pool(name="s_ex", bufs=1))
        s_exb = ctx.enter_context(tc.tile_pool(name="s_exb", bufs=1))
        s_sm = ctx.enter_context(tc.tile_pool(name="s_sm", bufs=1))
        s_h = ctx.enter_context(tc.tile_pool(name="s_h", bufs=2))
        s_o = ctx.enter_context(tc.tile_pool(name="s_o", bufs=1))

        def wload(dram, shape, dt=BF16):
            t = wp.tile(list(shape), dt, tag=dram.name, name=dram.name)
            nc.sync.dma_start(t[:], dram[:])
            return t

        ownT_s = wload(ownT, [OWN_D + 1, bc])
        ownW_s = wload(ownW, [OWN_D + 1, D])
        intW_s = wload(intW, [INT_D + 1, D])
        wq_s = wload(wq, [D, D])
        wk_s = wload(wk, [D, D])
        vattm_s = wload(vattm, [D, NI * NI])
        id32_s = wload(id32, [NI, NI])
        ones32_s = wload(ones32, [NI, 1])
        wvp_s = wload(wvp, [D, D])
        h1oe0_s = wload(h1oe0, [D, D]); h1oe1_s = wload(h1oe1, [D, D])
        h1at0_s = wload(h1at0, [D, D]); h1at1_s = wload(h1at1, [D, D])
        h2a0_s = wload(h2a0, [D, D]); h2a1_s = wload(h2a1, [D, D])
        h2b0_s = wload(h2b0, [D, D]); h2b1_s = wload(h2b1, [D, D])
        outlo_s = wload(outlo, [D, OUT_D]); outhi_s = wload(outhi, [D, OUT_D])
        br_s = wload(biasrows, [1, 6 * D])
        ones_s = wload(onesrow, [1, b_tile])

        tile_off = [0]
        for c in cs:
            tile_off.append(tile_off[-1] + c * b_tile)

        # ---------------- per-tile emission ----------------
        def dma_intr(t):
            c = cs[t]
            it = s_intr.tile([INT_D + 1, cmax * b_tile], BF16, tag="intr",
                             name="it")
            nc.sync.dma_start(it[:, 0:c * b_tile],
                              intrT[:, tile_off[t]:tile_off[t + 1]])
            mk = s_mask.tile([NI, b_tile], BF16, tag="mask", name="mk")
            nc.sync.dma_start(mk[:], maskT[t])
            return it, mk

        def emit_T(t, st, pending, pop):
            """T-phase for tile t. st carries tile state; pending/pop
            interleave closures from older tiles."""
            c = cs[t]
            s0 = t * b_tile
            it, mk = st["intr"]
            # own embedding
            poe = p_z.tile([D, b_tile], F32, tag="z", name="poe")
            nc.tensor.matmul(poe[:], ownW_s[:], ownT_s[:, s0:s0 + b_tile])
            oe = s_oe.tile([D, b_tile], BF16, tag="oe", name="oe")
            nc.scalar.activation(oe[:], poe[:], AF.Prelu, alpha=NEG_SLOPE)
            st["oe"] = oe
            ie = s_ie.tile([D, cmax * b_tile], BF16, tag="ie", name="ie")
            st["ie"] = ie
            sct = p_s.tile([NI, b_tile], F32, tag="sct", name="sct")
            st["sct"] = sct

            pairs = [(2 * j, min(2 * j + 1, c - 1)) for j in range((c + 1) // 2)]
            npairs = len(pairs)
            pech = {}
            echch = {}

            def z_lrelu(n):
                pz = p_z.tile([D, b_tile], F32, tag="z", name="pz")
                nc.tensor.matmul(
                    pz[:], intW_s[:],
                    it[:, n * b_tile:(n + 1) * b_tile])
                dst = ie[:, n * b_tile:(n + 1) * b_tile]
                r = _lrelu_route(n)
                if r == "A":
                    nc.scalar.activation(dst, pz[:], AF.Prelu,
                                         alpha=NEG_SLOPE)
                elif r == "D":
                    zs = s_zs.tile([D, b_tile], BF16, tag="zs", name="zs")
                    nc.vector.tensor_scalar_mul(zs[:], pz[:], NEG_SLOPE)
                    nc.vector.tensor_tensor(dst, zs[:], pz[:], op=ALU.max)
                else:  # fallback = D route
                    zs = s_zs.tile([D, b_tile], BF16, tag="zs", name="zs")
                    nc.vector.tensor_scalar_mul(zs[:], pz[:], NEG_SLOPE)
                    nc.vector.tensor_tensor(dst, zs[:], pz[:], op=ALU.max)

            def qk(jp):
                n0, n1 = pairs[jp]
                pe = p_e.tile([D, 2 * b_tile], F32, tag="e", name="pe")
                pech[jp] = pe
                for h, n in enumerate(dict.fromkeys((n0, n1))):
                    half = pe[:, h * b_tile:(h + 1) * b_tile]
                    src = ie[:, n * b_tile:(n + 1) * b_tile]
                    if Q_DVE_EVERY and n % Q_DVE_EVERY == Q_DVE_EVERY - 1:
                        nc.tensor.matmul(half, wk_s[:], src)
                        nc.vector.tensor_tensor(half, half, st["oe"][:],
                                                op=ALU.add)
                    else:
                        nc.tensor.matmul(half, wk_s[:], src,
                                         start=True, stop=False)
                        nc.tensor.matmul(half, wq_s[:], st["oe"][:],
                                         start=False, stop=True)
                w = b_tile if n1 == n0 else 2 * b_tile
                ech = s_ech.tile([D, 2 * b_tile], BF16, tag="ech", name="ech")
                nc.scalar.activation(ech[:, 0:w], pe[:, 0:w], AF.Tanh)
                echch[jp] = ech

            def sc(jp):
                n0, n1 = pairs[jp]
                ech = echch.pop(jp)
                for h, n in enumerate(dict.fromkeys((n0, n1))):
                    nc.tensor.matmul(
                        sct[:], vattm_s[:, n * NI:(n + 1) * NI],
                        ech[:, h * b_tile:(h + 1) * b_tile],
                        start=(n == 0), stop=False, skip_group_check=True)

            for j in range(npairs + 2):
                if j < npairs:
                    for n in dict.fromkeys(pairs[j]):
                        z_lrelu(n)
                if 1 <= j <= npairs:
                    qk(j - 1)
                if 2 <= j <= npairs + 1:
                    sc(j - 2)
                pop(pending)
            # pad-slot mask (exact also when no pads: mask rows are 0)
            nc.tensor.matmul(sct[:], id32_s[0:c, :], mk[0:c, :],
                             start=False, stop=True, skip_group_check=True)

        def steps_A(t, st):
            """Attention phase closures for tile t (run during t+1)."""
            c = cs[t]
            box = {}

            def s_exp():
                ex = s_ex.tile([NI, b_tile], BF16, tag="ex", name="ex")
                nc.scalar.activation(ex[0:c, :], st["sct"][0:c, :], AF.Exp)
                box["ex"] = ex

            def s_sum():
                ps = p_m.tile([D, b_tile], F32, tag="pm", name="ps")
                nc.tensor.matmul(ps[0:1, :], ones32_s[0:c, 0:1],
                                 box["ex"][0:c, :])
                rs = s_sm.tile([1, b_tile], F32, tag="rs", name="rs")
                nc.vector.reciprocal(rs[:], ps[0:1, :])
                rb = s_sm.tile([1, b_tile], BF16, tag="rb", name="rb")
                nc.vector.tensor_copy(rb[:], rs[:])
                box["rb"] = rb

            def s_norm():
                rb32 = s_sm.tile([NI, b_tile], BF16, tag="rb32", name="rb32")
                nc.gpsimd.partition_broadcast(rb32[0:c, :], box["rb"][0:1, :],
                                              channels=c)
                exn = s_ex.tile([NI, b_tile], BF16, tag="exn", name="exn")
                nc.vector.tensor_tensor(exn[0:c, :], box["ex"][0:c, :],
                                        rb32[0:c, :], op=ALU.mult)
                nc.sync.dma_start(exd[t][0:c * b_tile],
                                  exn[0:c, :])
                box["exn"] = exn

            def s_bcast(g):
                def f():
                    if "exb" not in box:
                        box["exb"] = s_exb.tile([D, cmax * b_tile], BF16,
                                                tag="exb", name="exb")
                    exb = box["exb"]
                    lo = g * EXB_GROUP
                    hi = min(c, lo + EXB_GROUP)
                    w = (hi - lo) * b_tile
                    src = AP(exd, t * NI * b_tile + lo * b_tile,
                             [[0, D], [1, w]])
                    nc.sync.dma_start(
                        exb[:, lo * b_tile:lo * b_tile + w], src)
                return f

            def s_mul(n0, n1, k):
                def f():
                    # in-place: exb <- exb * ie (products overwrite alphas)
                    box["prod"] = box["exb"]
                    w = (n1 - n0) * b_tile
                    eng = (nc.gpsimd if MUL_POOL_EVERY and
                           k % MUL_POOL_EVERY == MUL_POOL_EVERY - 1
                           else nc.vector)
                    eng.tensor_tensor(
                        box["exb"][:, n0 * b_tile:n0 * b_tile + w],
                        box["exb"][:, n0 * b_tile:n0 * b_tile + w],
                        st["ie"][:, n0 * b_tile:n0 * b_tile + w],
                        op=ALU.mult)
                return f

            def s_tree(width, rem, half):
                # prod[:, 0:half*b] += prod[:, rem*b : (rem+half)*b]
                def f():
                    prod = box["prod"]
                    pw = int(half * b_tile * TREE_POOL_FRAC) & ~1
                    dw = half * b_tile - pw
                    dst = prod[:, 0:half * b_tile]
                    srcl = prod[:, rem * b_tile:rem * b_tile + dw]
                    if dw:
                        nc.vector.tensor_tensor(
                            prod[:, 0:dw], prod[:, 0:dw], srcl, op=ALU.add)
                    if pw:
                        nc.gpsimd.tensor_tensor(
                            prod[:, dw:dw + pw], prod[:, dw:dw + pw],
                            prod[:, rem * b_tile + dw:
                                 rem * b_tile + dw + pw], op=ALU.add)
                return f

            steps = [s_exp, s_sum, s_norm]
            ngroups = (c + EXB_GROUP - 1) // EXB_GROUP
            mul_plan = []
            for g in range(ngroups):
                steps.append(s_bcast(g))
                lo, hi = g * EXB_GROUP, min(c, (g + 1) * EXB_GROUP)
                # muls in [1024]-wide ops
                n = lo
                while n < hi:
                    n2 = min(n + 2, hi)
                    steps.append(s_mul(n, n2, len(mul_plan)))
                    mul_plan.append(n)
                    n = n2
            w = c
            while w > 1:
                half = w // 2
                rem = w - half
                steps.append(s_tree(w, rem, half))
                w = rem
            st["box"] = box
            return steps

        def steps_H(t, st):
            """Head closures for tile t (ctxpre = prod[:, 0:b_tile])."""
            c = cs[t]
            s0 = t * b_tile
            box = st["box"]
            hb = {}

            def bias_mm(ph, k):
                # ph += biasrow_k^T @ onesrow  (only when biases nonzero)
                if with_bias_mms:
                    nc.tensor.matmul(ph, br_s[0:1, k * D:(k + 1) * D],
                                     ones_s[0:1, :],
                                     start=False, stop=True,
                                     skip_group_check=True)

            def mm2(w0, in0, w1, in1, k, cols=D):
                ph = p_m.tile([D, b_tile], F32, tag="pm", name="ph")
                nc.tensor.matmul(ph[0:cols, :], w0[:, 0:cols], in0,
                                 start=True, stop=False,
                                 skip_group_check=True)
                nc.tensor.matmul(ph[0:cols, :], w1[:, 0:cols], in1,
                                 start=False, stop=(not with_bias_mms),
                                 skip_group_check=True)
                bias_mm(ph[0:cols, :], k)
                return ph

            def l_attn():
                ph = p_m.tile([D, b_tile], F32, tag="pm", name="ph")
                nc.tensor.matmul(ph[:], wvp_s[:], box["prod"][:, 0:b_tile],
                                 start=True, stop=(not with_bias_mms),
                                 skip_group_check=True)
                bias_mm(ph[:], 0)
                at = s_h.tile([D, b_tile], BF16, tag="attn", name="at")
                nc.scalar.activation(at[:], ph[:], AF.Tanh)
                hb["attn"] = at

            def l_h1(i):
                def f():
                    ph = mm2(h1oe0_s if i == 0 else h1oe1_s, st["oe"][:],
                             h1at0_s if i == 0 else h1at1_s, hb["attn"][:],
                             1 + i)
                    hh = s_h.tile([D, b_tile], BF16, tag=f"h1{i}", name="hh")
                    nc.scalar.activation(hh[:], ph[:], AF.Prelu,
                                         alpha=NEG_SLOPE)
                    hb[f"h1{i}"] = hh
                return f

            def l_h2(i):
                def f():
                    ph = mm2(h2a0_s if i == 0 else h2a1_s, hb["h10"][:],
                             h2b0_s if i == 0 else h2b1_s, hb["h11"][:],
                             3 + i)
                    hh = s_h.tile([D, b_tile], BF16, tag=f"h2{i}", name="hh")
                    nc.scalar.activation(hh[:], ph[:], AF.Prelu,
                                         alpha=NEG_SLOPE)
                    hb[f"h2{i}"] = hh
                return f

            def l_out():
                ph = p_m.tile([D, b_tile], F32, tag="pm", name="ph")
                nc.tensor.matmul(ph[0:OUT_D, :], outlo_s[:], hb["h20"][:],
                                 start=True, stop=False,
                                 skip_group_check=True)
                nc.tensor.matmul(ph[0:OUT_D, :], outhi_s[:], hb["h21"][:],
                                 start=False, stop=(not with_bias_mms),
                                 skip_group_check=True)
                if with_bias_mms:
                    nc.tensor.matmul(ph[0:OUT_D, :],
                                     br_s[0:1, 5 * D:5 * D + OUT_D],
                                     ones_s[0:1, :], start=False, stop=True,
                                     skip_group_check=True)
                osb = s_o.tile([OUT_D, b_tile], F32, tag="o", name="osb")
                nc.vector.tensor_copy(osb[:], ph[0:OUT_D, :])
                nc.sync.dma_start(y[:, s0:s0 + b_tile], osb[:])

            return [l_attn, l_h1(0), l_h1(1), l_h2(0), l_h2(1), l_out]

        # ---------------- pipeline ----------------
        def pop_factory(total_chunks):
            state = {"i": 0}

            def pop(pending):
                state["i"] += 1
                left = total_chunks - state["i"]
                if left <= 0:
                    k = len(pending)
                else:
                    k = -(-len(pending) // (left + 1))
                for _ in range(min(k, len(pending))):
                    pending.pop(0)()
            return pop

        pending = []
        st_prev = None
        intr_next = dma_intr(0)
        for t in range(nt):
            st = {"intr": intr_next}
            if t + 1 < nt:
                intr_next = dma_intr(t + 1)
            nchunks = (cs[t] + 1) // 2 + 2
            emit_T(t, st, pending, pop_factory(nchunks))
            assert not pending
            pending = steps_A(t, st) + steps_H(t, st)
            st_prev = st
        for f in pending:
            f()

    nc.compile()
    return nc


# ---------------- host-side prep ----------------

_CACHED = {}


def prep_inputs(obs, own_W, own_b, int_W, int_b, Wq, Wk, Wv, v_att,
                proj_W, proj_b, h1_W, h1_b, h2_W, h2_b, out_W, out_b):
    obs = np.asarray(obs, np.float32)
    B = obs.shape[0]
    assert B == B_FULL
    f32 = lambda a: np.ascontiguousarray(np.asarray(a, np.float32))
    bf = lambda a: np.ascontiguousarray(
        np.asarray(a, np.float32).astype(BF16_NP))

    own = obs[:, :OWN_D]
    intr = obs[:, OWN_D:].reshape(B, NI, INT_D)
    live = np.abs(intr).sum(2) >= 1e-6          # [B, NI]
    cnt = live.sum(1).astype(np.int64)          # [B]
    assert cnt.min() >= 1, "all-padded sample: compaction unsupported"

    order = np.argsort(cnt, kind="stable")
    nt = NT
    # tile position j on every core uses c*_j = max count in global group j
    cs = tuple(int(cnt[order[(8 * j + 8) * B_TILE - 1]]) for j in range(nt))

    # live-first column permutation per sample
    key = np.argsort(~live, axis=1, kind="stable")      # [B, NI]

    with_bias = any(np.any(np.asarray(b)) for b in
                    (proj_b, h1_b, h2_b, out_b))

    h1_W = np.asarray(h1_W, np.float32)
    h2_W = np.asarray(h2_W, np.float32)
    out_W = np.asarray(out_W, np.float32)
    Wv = np.asarray(Wv, np.float32)
    proj_W = np.asarray(proj_W, np.float32)
    vattm = np.zeros((D, NI, NI), np.float32)
    for n in range(NI):
        vattm[:, n, n] = np.asarray(v_att, np.float32)
    biasrows = np.zeros((1, 6 * D), np.float32)
    biasrows[0, 0:D] = np.asarray(proj_b, np.float32)
    biasrows[0, D:2 * D] = np.asarray(h1_b, np.float32)[:D]
    biasrows[0, 2 * D:3 * D] = np.asarray(h1_b, np.float32)[D:]
    biasrows[0, 3 * D:4 * D] = np.asarray(h2_b, np.float32)[:D]
    biasrows[0, 4 * D:5 * D] = np.asarray(h2_b, np.float32)[D:]
    biasrows[0, 5 * D:5 * D + OUT_D] = np.asarray(out_b, np.float32)

    shared = dict(
        ownW=bf(np.concatenate([np.asarray(own_W, np.float32),
                                np.asarray(own_b, np.float32)[None]], 0)),
        intW=bf(np.concatenate([np.asarray(int_W, np.float32),
                                np.asarray(int_b, np.float32)[None]], 0)),
        wq=bf(Wq), wk=bf(Wk),
        vattm=bf(vattm.reshape(D, NI * NI)),
        id32=bf(np.eye(NI)),
        ones32=bf(np.ones((NI, 1))),
        wvp=bf(Wv @ proj_W),
        h1oe0=bf(h1_W[:D, :D]), h1oe1=bf(h1_W[:D, D:]),
        h1at0=bf(h1_W[D:, :D]), h1at1=bf(h1_W[D:, D:]),
        h2a0=bf(h2_W[:D, :D]), h2a1=bf(h2_W[:D, D:]),
        h2b0=bf(h2_W[D:, :D]), h2b1=bf(h2_W[D:, D:]),
        outlo=bf(out_W[:D]), outhi=bf(out_W[D:]),
        biasrows=bf(biasrows),
        onesrow=bf(np.ones((1, B_TILE))),
    )

    in_maps = []
    perms = []
    for i in range(N_CORES):
        perm_i = np.concatenate(
            [order[(8 * j + i) * B_TILE:(8 * j + i + 1) * B_TILE]
             for j in range(nt)])
        perms.append(perm_i)
        intr_cols = []
        masks = np.zeros((nt, NI, B_TILE), np.float32)
        for j in range(nt):
            c = cs[j]
            idx = perm_i[j * B_TILE:(j + 1) * B_TILE]
            ic = np.take_along_axis(intr[idx], key[idx][:, :c, None],
                                    axis=1)          # [b, c, 7] live-first
            pad = np.arange(c)[None, :] >= cnt[idx][:, None]   # [b, c]
            ic = np.where(pad[:, :, None], 0.0, ic)
            # [8, c, b]: 7 features + ones row
            blk = np.concatenate(
                [ic.transpose(2, 1, 0),
                 np.ones((1, c, B_TILE), np.float32)], 0)
            intr_cols.append(blk.reshape(INT_D + 1, c * B_TILE))
            masks[j, :c, :] = np.where(pad.T, NEG_BIG, 0.0)
        ownT_i = np.concatenate(
            [own[perm_i].T, np.ones((1, BC), np.float32)], 0)
        in_maps.append(dict(
            shared,
            intrT=bf(np.concatenate(intr_cols, 1)),
            ownT=bf(ownT_i),
            maskT=bf(masks),
        ))

    _CACHED["cs"] = cs
    _CACHED["perms"] = perms
    _CACHED["with_bias"] = with_bias
    key_ = (cs, with_bias)
    if _CACHED.get("key") != key_:
        _CACHED["nc"] = build_program(cs, with_bias_mms=with_bias)
        _CACHED["key"] = key_
    return in_maps


def _get_program():
    if "nc" not in _CACHED:
        # default program (uniform worst case) for standalone inspection
        _CACHED["nc"] = build_program(tuple([NI] * NT))
        _CACHED["key"] = (tuple([NI] * NT), False)
    return _CACHED["nc"]


def run_on_device(in_maps, trace=False):
    from concourse.bass_utils import run_bass_kernel_spmd
    nc = _get_program()
    return run_bass_kernel_spmd(nc, in_maps,
                                core_ids=list(range(len(in_maps))),
                                trace=trace)


def postprocess(res):
    B = B_FULL
    out = np.empty((B, OUT_D), np.float32)
    for i, r in enumerate(res.results):
        out[_CACHED["perms"][i]] = np.asarray(r["y"]).T
    return out


def kernel(**inputs):
    in_maps = prep_inputs(**inputs)
    try:
        res = run_on_device(in_maps)
    except Exception:
        import time as _time
        _time.sleep(10)
        res = run_on_device(in_maps)
    return postprocess(res)


# revision 6
# speedup vs baseline: 1.0110x; 1.0110x over previous
"""Trainium2 Bass kernel for AdditiveAttentionSACModel (v2).

Data-parallel over 8 NeuronCores, BC=4096 samples/core, 8 tiles of 512.
Feature-major on-chip layout: ATTN_D=128 on partitions, (intruder, sample)
tokens on the free dim.

Key structural choices (vs the v1 kernel this replaces):
  - Host-side compaction: samples are globally sorted by live-intruder
    count and padded slots are dropped; each tile processes only
    c_t = max live count in tile slots (~76% of the 32 slots on average).
    Pad residue inside a tile is masked with a -1e30 additive matmul.
  - Softmax without transposes or max-subtraction: scores stay [32, b],
    sum over slots via a ones-matmul (contracting only c rows), exp has no
    overflow risk (|scores| <= sum|v_att| ~ 14).
  - Alpha broadcast to 128 partitions via DRAM round-trip DMA (idle
    fabric), normalized alphas multiply ie on DVE in 2x bf16 mode;
    products reduce with a halving tree split across DVE and Pool.
  - Wv @ proj_W precombined on host: ctx/proj collapse into one matmul.
  - lrelu evacuations of z are load-balanced across Act (1 op Prelu),
    DVE (2 ops), and DVE-copy+Pool-stt routes.
  - y is produced feature-major [2, bc] and fixed up on host.
Weights/biases: own/int biases ride a constant-one feature row (exact for
any values); other biases are all zero in this model - the host checks and
falls back to extra bias matmuls if they are ever nonzero.
"""

import numpy as np
import ml_dtypes

import concourse.bass as bass
import concourse.bacc as bacc
import concourse.mybir as mybir
import concourse.tile as tile
from concourse.ap import AP
from contextlib import ExitStack

# ---- problem constants (hardcoded; kernel.py must be self-contained) ----
N_CORES = 8
B_FULL = 32768
BC = B_FULL // N_CORES          # 4096 samples per core
NI = 32
OWN_D = 3
INT_D = 7
D = 128
HID = 256
OUT_D = 2
OBS_D = OWN_D + NI * INT_D      # 227
NEG_SLOPE = 0.2

B_TILE = 512
NT = BC // B_TILE               # 8 tiles per core
NEG_BIG = -1.0e30

F32 = mybir.dt.float32
BF16 = mybir.dt.bfloat16
AF = mybir.ActivationFunctionType
ALU = mybir.AluOpType
BF16_NP = ml_dtypes.bfloat16

# ---- tuning knobs ----
# lrelu evac route per slot index: 'A' Act Prelu, 'D' DVE 2-op,
# 'P' DVE copy + Pool scalar_tensor_tensor
LRELU_PATTERN = "AADAD"
Q_DVE_EVERY = 0      # every k-th q-add on DVE instead of PE (0 = none)
TREE_POOL_FRAC = 0.5   # column fraction of each tree level done on Pool
MUL_POOL_EVERY = 3   # every k-th alpha-mul op on Pool (0 = none)
EXB_GROUP = 8        # slots per alpha-broadcast DMA


def _lrelu_route(n):
    return LRELU_PATTERN[n % len(LRELU_PATTERN)]


def build_program(cs, bc=BC, b_tile=B_TILE, with_bias_mms=False):
    """Per-core Bass program. cs = per-tile live-slot counts (same on all
    cores; SPMD). with_bias_mms adds explicit bias matmuls for the head
    (needed only if proj/h1/h2/out biases are nonzero)."""
    nt = len(cs)
    assert nt * b_tile == bc
    cmax = max(cs)
    tot_cols = sum(c * b_tile for c in cs)

    nc = bacc.Bacc("TRN2", target_bir_lowering=False, debug=False,
                   num_devices=N_CORES)

    def din(name, shape, dt=BF16):
        return nc.dram_tensor(name, list(shape), dt, kind="ExternalInput")

    intrT = din("intrT", [INT_D + 1, tot_cols])    # compacted tokens
    ownT = din("ownT", [OWN_D + 1, bc])
    maskT = din("maskT", [nt, NI, b_tile])          # -1e30 at pad slots
    ownW = din("ownW", [OWN_D + 1, D])
    intW = din("intW", [INT_D + 1, D])
    wq = din("wq", [D, D])
    wk = din("wk", [D, D])
    vattm = din("vattm", [D, NI * NI])
    id32 = din("id32", [NI, NI])
    ones32 = din("ones32", [NI, 1])
    wvp = din("wvp", [D, D])
    h1oe0 = din("h1oe0", [D, D]); h1oe1 = din("h1oe1", [D, D])
    h1at0 = din("h1at0", [D, D]); h1at1 = din("h1at1", [D, D])
    h2a0 = din("h2a0", [D, D]); h2a1 = din("h2a1", [D, D])
    h2b0 = din("h2b0", [D, D]); h2b1 = din("h2b1", [D, D])
    outlo = din("outlo", [D, OUT_D]); outhi = din("outhi", [D, OUT_D])
    biasrows = din("biasrows", [1, 6 * D])   # projb,h1b0,h1b1,h2b0,h2b1,outb
    onesrow = din("onesrow", [1, b_tile])

    exd = nc.dram_tensor("exd", [nt, NI * b_tile], BF16, kind="Internal")
    y = nc.dram_tensor("y", [OUT_D, bc], F32, kind="ExternalOutput")

    with tile.TileContext(nc) as tc, ExitStack() as ctx:
        # PSUM: pz 2 + pe 4 + sct 1 + pm 1 = 8 banks
        p_z = ctx.enter_context(tc.tile_pool(name="p_z", bufs=2, space="PSUM"))
        p_e = ctx.enter_context(tc.tile_pool(name="p_e", bufs=2, space="PSUM"))
        p_s = ctx.enter_context(tc.tile_pool(name="p_s", bufs=1, space="PSUM"))
        p_m = ctx.enter_context(tc.tile_pool(name="p_m", bufs=1, space="PSUM"))

        wp = ctx.enter_context(tc.tile_pool(name="wp", bufs=1))
        s_intr = ctx.enter_context(tc.tile_pool(name="s_intr", bufs=2))
        s_mask = ctx.enter_context(tc.tile_pool(name="s_mask", bufs=2))
        s_ie = ctx.enter_context(tc.tile_pool(name="s_ie", bufs=2))
        s_oe = ctx.enter_context(tc.tile_pool(name="s_oe", bufs=3))
        s_ech = ctx.enter_context(tc.tile_pool(name="s_ech", bufs=2))
        s_zs = ctx.enter_context(tc.tile_pool(name="s_zs", bufs=3))
        s_ex = ctx.enter_context(tc.tile_pool(name="s_ex", bufs=1))
        s_exb = ctx.enter_context(tc.tile_pool(name="s_exb", bufs=1))
        s_sm = ctx.enter_context(tc.tile_pool(name="s_sm", bufs=1))
        s_h = ctx.enter_context(tc.tile_pool(name="s_h", bufs=2))
        s_o = ctx.enter_context(tc.tile_pool(name="s_o", bufs=1))

        def wload(dram, shape, dt=BF16):
            t = wp.tile(list(shape), dt, tag=dram.name, name=dram.name)
            nc.sync.dma_start(t[:], dram[:])
            return t

        ownT_s = wload(ownT, [OWN_D + 1, bc])
        ownW_s = wload(ownW, [OWN_D + 1, D])
        intW_s = wload(intW, [INT_D + 1, D])
        wq_s = wload(wq, [D, D])
        wk_s = wload(wk, [D, D])
        vattm_s = wload(vattm, [D, NI * NI])
        id32_s = wload(id32, [NI, NI])
        ones32_s = wload(ones32, [NI, 1])
        wvp_s = wload(wvp, [D, D])
        h1oe0_s = wload(h1oe0, [D, D]); h1oe1_s = wload(h1oe1, [D, D])
        h1at0_s = wload(h1at0, [D, D]); h1at1_s = wload(h1at1, [D, D])
        h2a0_s = wload(h2a0, [D, D]); h2a1_s = wload(h2a1, [D, D])
        h2b0_s = wload(h2b0, [D, D]); h2b1_s = wload(h2b1, [D, D])
        outlo_s = wload(outlo, [D, OUT_D]); outhi_s = wload(outhi, [D, OUT_D])
        br_s = wload(biasrows, [1, 6 * D])
        ones_s = wload(onesrow, [1, b_tile])

        tile_off = [0]
        for c in cs:
            tile_off.append(tile_off[-1] + c * b_tile)

        # ---------------- per-tile emission ----------------
        def dma_intr(t):
            c = cs[t]
            it = s_intr.tile([INT_D + 1, cmax * b_tile], BF16, tag="intr",
                             name="it")
            nc.sync.dma_start(it[:, 0:c * b_tile],
                              intrT[:, tile_off[t]:tile_off[t + 1]])
            mk = s_mask.tile([NI, b_tile], BF16, tag="mask", name="mk")
            nc.sync.dma_start(mk[:], maskT[t])
            return it, mk

        def emit_T(t, st, pending, pop):
            """T-phase for tile t. st carries tile state; pending/pop
            interleave closures from older tiles."""
            c = cs[t]
            s0 = t * b_tile
            it, mk = st["intr"]
            # own embedding
            poe = p_z.tile([D, b_tile], F32, tag="z", name="poe")
            nc.tensor.matmul(poe[:], ownW_s[:], ownT_s[:, s0:s0 + b_tile])
            oe = s_oe.tile([D, b_tile], BF16, tag="oe", name="oe")
            nc.scalar.activation(oe[:], poe[:], AF.Prelu, alpha=NEG_SLOPE)
            st["oe"] = oe
            ie = s_ie.tile([D, cmax * b_tile], BF16, tag="ie", name="ie")
            st["ie"] = ie
            sct = p_s.tile([NI, b_tile], F32, tag="sct", name="sct")
            st["sct"] = sct

            pairs = [(2 * j, min(2 * j + 1, c - 1)) for j in range((c + 1) // 2)]
            npairs = len(pairs)
            pech = {}
            echch = {}

            def z_lrelu(n):
                pz = p_z.tile([D, b_tile], F32, tag="z", name="pz")
                nc.tensor.matmul(
                    pz[:], intW_s[:],
                    it[:, n * b_tile:(n + 1) * b_tile])
                dst = ie[:, n * b_tile:(n + 1) * b_tile]
                r = _lrelu_route(n)
                if r == "A":
                    nc.scalar.activation(dst, pz[:], AF.Prelu,
                                         alpha=NEG_SLOPE)
                elif r == "D":
                    zs = s_zs.tile([D, b_tile], BF16, tag="zs", name="zs")
                    nc.vector.tensor_scalar_mul(zs[:], pz[:], NEG_SLOPE)
                    nc.vector.tensor_tensor(dst, zs[:], pz[:], op=ALU.max)
                else:  # fallback = D route
                    zs = s_zs.tile([D, b_tile], BF16, tag="zs", name="zs")
                    nc.vector.tensor_scalar_mul(zs[:], pz[:], NEG_SLOPE)
                    nc.vector.tensor_tensor(dst, zs[:], pz[:], op=ALU.max)

            def qk(jp):
                n0, n1 = pairs[jp]
                pe = p_e.tile([D, 2 * b_tile], F32, tag="e", name="pe")
                pech[jp] = pe
                for h, n in enumerate(dict.fromkeys((n0, n1))):
                    half = pe[:, h * b_tile:(h + 1) * b_tile]
                    src = ie[:, n * b_tile:(n + 1) * b_tile]
                    if Q_DVE_EVERY and n % Q_DVE_EVERY == Q_DVE_EVERY - 1:
                        nc.tensor.matmul(half, wk_s[:], src)
                        nc.vector.tensor_tensor(half, half, st["oe"][:],
                                                op=ALU.add)
                    else:
                        nc.tensor.matmul(half, wk_s[:], src,
                                         start=True, stop=False)
                        nc.tensor.matmul(half, wq_s[:], st["oe"][:],
                                         start=False, stop=True)
                w = b_tile if n1 == n0 else 2 * b_tile
                ech = s_ech.tile([D, 2 * b_tile], BF16, tag="ech", name="ech")
                nc.scalar.activation(ech[:, 0:w], pe[:, 0:w], AF.Tanh)
                echch[jp] = ech

            def sc(jp):
                n0, n1 = pairs[jp]
                ech = echch.pop(jp)
                for h, n in enumerate(dict.fromkeys((n0, n1))):
                    nc.tensor.matmul(
                        sct[:], vattm_s[:, n * NI:(n + 1) * NI],
                        ech[:, h * b_tile:(h + 1) * b_tile],
                        start=(n == 0), stop=False, skip_group_check=True)

            for j in range(npairs + 2):
                if j < npairs:
                    for n in dict.fromkeys(pairs[j]):
                        z_lrelu(n)
                if 1 <= j <= npairs:
                    qk(j - 1)
                if 2 <= j <= npairs + 1:
                    sc(j - 2)
                pop()
            # pad-slot mask (exact also when no pads: mask rows are 0)
            nc.tensor.matmul(sct[:], id32_s[0:c, :], mk[0:c, :],
                             start=False, stop=True, skip_group_check=True)

        def steps_A(t, st):
            """Attention phase closures for tile t (run during t+1)."""
            c = cs[t]
            box = {}

            def s_exp():
                ex = s_ex.tile([NI, b_tile], BF16, tag="ex", name="ex")
                nc.scalar.activation(ex[0:c, :], st["sct"][0:c, :], AF.Exp)
                box["ex"] = ex

            def s_sum():
                ps = p_m.tile([D, b_tile], F32, tag="pm", name="ps")
                nc.tensor.matmul(ps[0:1, :], ones32_s[0:c, 0:1],
                                 box["ex"][0:c, :])
                rs = s_sm.tile([1, b_tile], F32, tag="rs", name="rs")
                nc.vector.reciprocal(rs[:], ps[0:1, :])
                rb = s_sm.tile([1, b_tile], BF16, tag="rb", name="rb")
                nc.vector.tensor_copy(rb[:], rs[:])
                box["rb"] = rb

            def s_norm():
                rb32 = s_sm.tile([NI, b_tile], BF16, tag="rb32", name="rb32")
                nc.gpsimd.partition_broadcast(rb32[0:c, :], box["rb"][0:1, :],
                                              channels=c)
                exn = s_ex.tile([NI, b_tile], BF16, tag="exn", name="exn")
                nc.vector.tensor_tensor(exn[0:c, :], box["ex"][0:c, :],
                                        rb32[0:c, :], op=ALU.mult)
                nc.sync.dma_start(exd[t][0:c * b_tile],
                                  exn[0:c, :])
                box["exn"] = exn

            def s_bcast(g):
                def f():
                    if "exb" not in box:
                        box["exb"] = s_exb.tile([D, cmax * b_tile], BF16,
                                                tag="exb", name="exb")
                    exb = box["exb"]
                    lo = g * EXB_GROUP
                    hi = min(c, lo + EXB_GROUP)
                    w = (hi - lo) * b_tile
                    src = AP(exd, t * NI * b_tile + lo * b_tile,
                             [[0, D], [1, w]])
                    nc.sync.dma_start(
                        exb[:, lo * b_tile:lo * b_tile + w], src)
                return f

            def s_mul(n0, n1, k):
                def f():
                    # in-place: exb <- exb * ie (products overwrite alphas)
                    box["prod"] = box["exb"]
                    w = (n1 - n0) * b_tile
                    eng = (nc.gpsimd if MUL_POOL_EVERY and
                           k % MUL_POOL_EVERY == MUL_POOL_EVERY - 1
                           else nc.vector)
                    eng.tensor_tensor(
                        box["exb"][:, n0 * b_tile:n0 * b_tile + w],
                        box["exb"][:, n0 * b_tile:n0 * b_tile + w],
                        st["ie"][:, n0 * b_tile:n0 * b_tile + w],
                        op=ALU.mult)
                return f

            def s_tree(width, rem, half):
                # prod[:, 0:half*b] += prod[:, rem*b : (rem+half)*b]
                def f():
                    prod = box["prod"]
                    pw = int(half * b_tile * TREE_POOL_FRAC) & ~1
                    dw = half * b_tile - pw
                    dst = prod[:, 0:half * b_tile]
                    srcl = prod[:, rem * b_tile:rem * b_tile + dw]
                    if dw:
                        nc.vector.tensor_tensor(
                            prod[:, 0:dw], prod[:, 0:dw], srcl, op=ALU.add)
                    if pw:
                        nc.gpsimd.tensor_tensor(
                            prod[:, dw:dw + pw], prod[:, dw:dw + pw],
                            prod[:, rem * b_tile + dw:
                                 rem * b_tile + dw + pw], op=ALU.add)
                return f

            steps = [(0, s_exp), (1, s_sum), (1, s_norm)]
            ngroups = (c + EXB_GROUP - 1) // EXB_GROUP
            mul_plan = []
            for g in range(ngroups):
                steps.append((2 + g, s_bcast(g)))
                lo, hi = g * EXB_GROUP, min(c, (g + 1) * EXB_GROUP)
                n = lo
                while n < hi:
                    n2 = min(n + 2, hi)
                    mul_plan.append((n, n2))
                    n = n2
            nm = len(mul_plan)
            for k, (n0, n1) in enumerate(mul_plan):
                steps.append((4 + (7 * k) // max(nm, 1), s_mul(n0, n1, k)))
            w = c
            lev = 0
            while w > 1:
                half = w // 2
                rem = w - half
                steps.append((11 + lev // 2, s_tree(w, rem, half)))
                lev += 1
                w = rem
            st["box"] = box
            return steps

        def steps_H(t, st):
            """Head closures for tile t (ctxpre = prod[:, 0:b_tile])."""
            c = cs[t]
            s0 = t * b_tile
            box = st["box"]
            hb = {}

            def bias_mm(ph, k):
                # ph += biasrow_k^T @ onesrow  (only when biases nonzero)
                if with_bias_mms:
                    nc.tensor.matmul(ph, br_s[0:1, k * D:(k + 1) * D],
                                     ones_s[0:1, :],
                                     start=False, stop=True,
                                     skip_group_check=True)

            def mm2(w0, in0, w1, in1, k, cols=D):
                ph = p_m.tile([D, b_tile], F32, tag="pm", name="ph")
                nc.tensor.matmul(ph[0:cols, :], w0[:, 0:cols], in0,
                                 start=True, stop=False,
                                 skip_group_check=True)
                nc.tensor.matmul(ph[0:cols, :], w1[:, 0:cols], in1,
                                 start=False, stop=(not with_bias_mms),
                                 skip_group_check=True)
                bias_mm(ph[0:cols, :], k)
                return ph

            def l_attn():
                ph = p_m.tile([D, b_tile], F32, tag="pm", name="ph")
                nc.tensor.matmul(ph[:], wvp_s[:], box["prod"][:, 0:b_tile],
                                 start=True, stop=(not with_bias_mms),
                                 skip_group_check=True)
                bias_mm(ph[:], 0)
                at = s_h.tile([D, b_tile], BF16, tag="attn", name="at")
                nc.scalar.activation(at[:], ph[:], AF.Tanh)
                hb["attn"] = at

            def l_h1(i):
                def f():
                    ph = mm2(h1oe0_s if i == 0 else h1oe1_s, st["oe"][:],
                             h1at0_s if i == 0 else h1at1_s, hb["attn"][:],
                             1 + i)
                    hh = s_h.tile([D, b_tile], BF16, tag=f"h1{i}", name="hh")
                    nc.scalar.activation(hh[:], ph[:], AF.Prelu,
                                         alpha=NEG_SLOPE)
                    hb[f"h1{i}"] = hh
                return f

            def l_h2(i):
                def f():
                    ph = mm2(h2a0_s if i == 0 else h2a1_s, hb["h10"][:],
                             h2b0_s if i == 0 else h2b1_s, hb["h11"][:],
                             3 + i)
                    hh = s_h.tile([D, b_tile], BF16, tag=f"h2{i}", name="hh")
                    nc.scalar.activation(hh[:], ph[:], AF.Prelu,
                                         alpha=NEG_SLOPE)
                    hb[f"h2{i}"] = hh
                return f

            def l_out():
                ph = p_m.tile([D, b_tile], F32, tag="pm", name="ph")
                nc.tensor.matmul(ph[0:OUT_D, :], outlo_s[:], hb["h20"][:],
                                 start=True, stop=False,
                                 skip_group_check=True)
                nc.tensor.matmul(ph[0:OUT_D, :], outhi_s[:], hb["h21"][:],
                                 start=False, stop=(not with_bias_mms),
                                 skip_group_check=True)
                if with_bias_mms:
                    nc.tensor.matmul(ph[0:OUT_D, :],
                                     br_s[0:1, 5 * D:5 * D + OUT_D],
                                     ones_s[0:1, :], start=False, stop=True,
                                     skip_group_check=True)
                osb = s_o.tile([OUT_D, b_tile], F32, tag="o", name="osb")
                nc.vector.tensor_copy(osb[:], ph[0:OUT_D, :])
                nc.sync.dma_start(y[:, s0:s0 + b_tile], osb[:])

            return [(13, l_attn), (13, l_h1(0)), (14, l_h1(1)),
                    (14, l_h2(0)), (15, l_h2(1)), (15, l_out)]

        # ---------------- pipeline (global chunk scheduler) ----------------
        sched = {"G": 0, "pending": []}

        def pop():
            sched["G"] += 1
            g = sched["G"]
            rest, run = [], []
            for due, f in sched["pending"]:
                (run if due <= g else rest).append((due, f))
            sched["pending"] = rest
            for _, f in sorted(run, key=lambda x: x[0]):
                f()

        intr_next = dma_intr(0)
        for t in range(nt):
            st = {"intr": intr_next}
            if t + 1 < nt:
                intr_next = dma_intr(t + 1)
            g0 = sched["G"]
            emit_T(t, st, None, pop)
            for due, f in steps_A(t, st) + steps_H(t, st):
                sched["pending"].append((sched["G"] + due + 1, f))
        for _, f in sorted(sched["pending"], key=lambda x: x[0]):
            f()

    nc.compile()
    return nc


# ---------------- host-side prep ----------------

_CACHED = {}


def prep_inputs(obs, own_W, own_b, int_W, int_b, Wq, Wk, Wv, v_att,
                proj_W, proj_b, h1_W, h1_b, h2_W, h2_b, out_W, out_b):
    obs = np.asarray(obs, np.float32)
    B = obs.shape[0]
    assert B == B_FULL
    f32 = lambda a: np.ascontiguousarray(np.asarray(a, np.float32))
    bf = lambda a: np.ascontiguousarray(
        np.asarray(a, np.float32).astype(BF16_NP))

    own = obs[:, :OWN_D]
    intr = obs[:, OWN_D:].reshape(B, NI, INT_D)
    live = np.abs(intr).sum(2) >= 1e-6          # [B, NI]
    cnt = live.sum(1).astype(np.int64)          # [B]
    assert cnt.min() >= 1, "all-padded sample: compaction unsupported"

    order = np.argsort(cnt, kind="stable")
    nt = NT
    # tile position j on every core uses c*_j = max count in global group j
    cs = tuple(int(cnt[order[(8 * j + 8) * B_TILE - 1]]) for j in range(nt))

    # live-first column permutation per sample
    key = np.argsort(~live, axis=1, kind="stable")      # [B, NI]

    with_bias = any(np.any(np.asarray(b)) for b in
                    (proj_b, h1_b, h2_b, out_b))

    h1_W = np.asarray(h1_W, np.float32)
    h2_W = np.asarray(h2_W, np.float32)
    out_W = np.asarray(out_W, np.float32)
    Wv = np.asarray(Wv, np.float32)
    proj_W = np.asarray(proj_W, np.float32)
    vattm = np.zeros((D, NI, NI), np.float32)
    for n in range(NI):
        vattm[:, n, n] = np.asarray(v_att, np.float32)
    biasrows = np.zeros((1, 6 * D), np.float32)
    biasrows[0, 0:D] = np.asarray(proj_b, np.float32)
    biasrows[0, D:2 * D] = np.asarray(h1_b, np.float32)[:D]
    biasrows[0, 2 * D:3 * D] = np.asarray(h1_b, np.float32)[D:]
    biasrows[0, 3 * D:4 * D] = np.asarray(h2_b, np.float32)[:D]
    biasrows[0, 4 * D:5 * D] = np.asarray(h2_b, np.float32)[D:]
    biasrows[0, 5 * D:5 * D + OUT_D] = np.asarray(out_b, np.float32)

    shared = dict(
        ownW=bf(np.concatenate([np.asarray(own_W, np.float32),
                                np.asarray(own_b, np.float32)[None]], 0)),
        intW=bf(np.concatenate([np.asarray(int_W, np.float32),
                                np.asarray(int_b, np.float32)[None]], 0)),
        wq=bf(Wq), wk=bf(Wk),
        vattm=bf(vattm.reshape(D, NI * NI)),
        id32=bf(np.eye(NI)),
        ones32=bf(np.ones((NI, 1))),
        wvp=bf(Wv @ proj_W),
        h1oe0=bf(h1_W[:D, :D]), h1oe1=bf(h1_W[:D, D:]),
        h1at0=bf(h1_W[D:, :D]), h1at1=bf(h1_W[D:, D:]),
        h2a0=bf(h2_W[:D, :D]), h2a1=bf(h2_W[:D, D:]),
        h2b0=bf(h2_W[D:, :D]), h2b1=bf(h2_W[D:, D:]),
        outlo=bf(out_W[:D]), outhi=bf(out_W[D:]),
        biasrows=bf(biasrows),
        onesrow=bf(np.ones((1, B_TILE))),
    )

    in_maps = []
    perms = []
    for i in range(N_CORES):
        perm_i = np.concatenate(
            [order[(8 * j + i) * B_TILE:(8 * j + i + 1) * B_TILE]
             for j in range(nt)])
        perms.append(perm_i)
        intr_cols = []
        masks = np.zeros((nt, NI, B_TILE), np.float32)
        for j in range(nt):
            c = cs[j]
            idx = perm_i[j * B_TILE:(j + 1) * B_TILE]
            ic = np.take_along_axis(intr[idx], key[idx][:, :c, None],
                                    axis=1)          # [b, c, 7] live-first
            pad = np.arange(c)[None, :] >= cnt[idx][:, None]   # [b, c]
            ic = np.where(pad[:, :, None], 0.0, ic)
            # [8, c, b]: 7 features + ones row
            blk = np.concatenate(
                [ic.transpose(2, 1, 0),
                 np.ones((1, c, B_TILE), np.float32)], 0)
            intr_cols.append(blk.reshape(INT_D + 1, c * B_TILE))
            masks[j, :c, :] = np.where(pad.T, NEG_BIG, 0.0)
        ownT_i = np.concatenate(
            [own[perm_i].T, np.ones((1, BC), np.float32)], 0)
        in_maps.append(dict(
            shared,
            intrT=bf(np.concatenate(intr_cols, 1)),
            ownT=bf(ownT_i),
            maskT=bf(masks),
        ))

    _CACHED["cs"] = cs
    _CACHED["perms"] = perms
    _CACHED["with_bias"] = with_bias
    key_ = (cs, with_bias)
    if _CACHED.get("key") != key_:
        _CACHED["nc"] = build_program(cs, with_bias_mms=with_bias)
        _CACHED["key"] = key_
    return in_maps


def _get_program():
    if "nc" not in _CACHED:
        # default program (uniform worst case) for standalone inspection
        _CACHED["nc"] = build_program(tuple([NI] * NT))
        _CACHED["key"] = (tuple([NI] * NT), False)
    return _CACHED["nc"]


def run_on_device(in_maps, trace=False):
    from concourse.bass_utils import run_bass_kernel_spmd
    nc = _get_program()
    return run_bass_kernel_spmd(nc, in_maps,
                                core_ids=list(range(len(in_maps))),
                                trace=trace)


def postprocess(res):
    B = B_FULL
    out = np.empty((B, OUT_D), np.float32)
    for i, r in enumerate(res.results):
        out[_CACHED["perms"][i]] = np.asarray(r["y"]).T
    return out


def kernel(**inputs):
    in_maps = prep_inputs(**inputs)
    try:
        res = run_on_device(in_maps)
    except Exception:
        import time as _time
        _time.sleep(10)
        res = run_on_device(in_maps)
    return postprocess(res)


# revision 7
# speedup vs baseline: 1.1264x; 1.1141x over previous
"""Trainium2 Bass kernel for AdditiveAttentionSACModel (v2).

Data-parallel over 8 NeuronCores, BC=4096 samples/core, 8 tiles of 512.
Feature-major on-chip layout: ATTN_D=128 on partitions, (intruder, sample)
tokens on the free dim.

Key structural choices (vs the v1 kernel this replaces):
  - Host-side compaction: samples are globally sorted by live-intruder
    count and padded slots are dropped; each tile processes only
    c_t = max live count in tile slots (~76% of the 32 slots on average).
    Pad residue inside a tile is masked with a -1e30 additive matmul.
  - Softmax without transposes or max-subtraction: scores stay [32, b],
    sum over slots via a ones-matmul (contracting only c rows), exp has no
    overflow risk (|scores| <= sum|v_att| ~ 14).
  - Alpha broadcast to 128 partitions via DRAM round-trip DMA (idle
    fabric), normalized alphas multiply ie on DVE in 2x bf16 mode;
    products reduce with a halving tree split across DVE and Pool.
  - Wv @ proj_W precombined on host: ctx/proj collapse into one matmul.
  - lrelu evacuations of z are load-balanced across Act (1 op Prelu),
    DVE (2 ops), and DVE-copy+Pool-stt routes.
  - y is produced feature-major [2, bc] and fixed up on host.
Weights/biases: own/int biases ride a constant-one feature row (exact for
any values); other biases are all zero in this model - the host checks and
falls back to extra bias matmuls if they are ever nonzero.
"""

import numpy as np
import ml_dtypes

import concourse.bass as bass
import concourse.bacc as bacc
import concourse.mybir as mybir
import concourse.tile as tile
from concourse.ap import AP
from contextlib import ExitStack

# ---- problem constants (hardcoded; kernel.py must be self-contained) ----
N_CORES = 8
B_FULL = 32768
BC = B_FULL // N_CORES          # 4096 samples per core
NI = 32
OWN_D = 3
INT_D = 7
D = 128
HID = 256
OUT_D = 2
OBS_D = OWN_D + NI * INT_D      # 227
NEG_SLOPE = 0.2

B_TILE = 512
NT = BC // B_TILE               # 8 tiles per core
NEG_BIG = -1.0e30

F32 = mybir.dt.float32
BF16 = mybir.dt.bfloat16
AF = mybir.ActivationFunctionType
ALU = mybir.AluOpType
BF16_NP = ml_dtypes.bfloat16

# ---- tuning knobs ----
# lrelu evac route per slot index: 'A' Act Prelu, 'D' DVE 2-op,
# 'P' DVE copy + Pool scalar_tensor_tensor
LRELU_PATTERN = "AADAD"
Q_DVE_EVERY = 0      # every k-th q-add on DVE instead of PE (0 = none)
TREE_POOL_FRAC = 0.5   # column fraction of each tree level done on Pool
MUL_POOL_EVERY = 3   # every k-th alpha-mul op on Pool (0 = none)
EXB_GROUP = 8        # slots per alpha-broadcast DMA


def _lrelu_route(n):
    return LRELU_PATTERN[n % len(LRELU_PATTERN)]


def build_program(cs, bc=BC, b_tile=B_TILE, with_bias_mms=False):
    """Per-core Bass program. cs = per-tile live-slot counts (same on all
    cores; SPMD). with_bias_mms adds explicit bias matmuls for the head
    (needed only if proj/h1/h2/out biases are nonzero)."""
    nt = len(cs)
    assert nt * b_tile == bc
    cmax = max(cs)
    tot_cols = sum(c * b_tile for c in cs)

    nc = bacc.Bacc("TRN2", target_bir_lowering=False, debug=False,
                   num_devices=N_CORES)

    def din(name, shape, dt=BF16):
        return nc.dram_tensor(name, list(shape), dt, kind="ExternalInput")

    intrT = din("intrT", [INT_D + 1, tot_cols])    # compacted tokens
    ownT = din("ownT", [OWN_D + 1, bc])
    maskT = din("maskT", [nt, NI, b_tile])          # -1e30 at pad slots
    ownW = din("ownW", [OWN_D + 1, D])
    intW = din("intW", [INT_D + 1, D])
    wq = din("wq", [D, D])
    wk = din("wk", [D, D])
    vattm = din("vattm", [D, NI * NI])
    id32 = din("id32", [NI, NI])
    ones32 = din("ones32", [NI, 1])
    wvp = din("wvp", [D, D])
    h1oe0 = din("h1oe0", [D, D]); h1oe1 = din("h1oe1", [D, D])
    h1at0 = din("h1at0", [D, D]); h1at1 = din("h1at1", [D, D])
    h2a0 = din("h2a0", [D, D]); h2a1 = din("h2a1", [D, D])
    h2b0 = din("h2b0", [D, D]); h2b1 = din("h2b1", [D, D])
    outlo = din("outlo", [D, OUT_D]); outhi = din("outhi", [D, OUT_D])
    biasrows = din("biasrows", [1, 6 * D])   # projb,h1b0,h1b1,h2b0,h2b1,outb
    onesrow = din("onesrow", [1, b_tile])

    exd = nc.dram_tensor("exd", [nt, NI * b_tile], BF16, kind="Internal")
    y = nc.dram_tensor("y", [OUT_D, bc], F32, kind="ExternalOutput")

    with tile.TileContext(nc) as tc, ExitStack() as ctx:
        # PSUM: pz 2 + pe 4 + sct 1 + pm 1 = 8 banks
        p_z = ctx.enter_context(tc.tile_pool(name="p_z", bufs=2, space="PSUM"))
        p_e = ctx.enter_context(tc.tile_pool(name="p_e", bufs=2, space="PSUM"))
        p_s = ctx.enter_context(tc.tile_pool(name="p_s", bufs=1, space="PSUM"))
        p_m = ctx.enter_context(tc.tile_pool(name="p_m", bufs=1, space="PSUM"))

        wp = ctx.enter_context(tc.tile_pool(name="wp", bufs=1))
        s_intr = ctx.enter_context(tc.tile_pool(name="s_intr", bufs=2))
        s_mask = ctx.enter_context(tc.tile_pool(name="s_mask", bufs=2))
        s_ie = ctx.enter_context(tc.tile_pool(name="s_ie", bufs=2))
        s_oe = ctx.enter_context(tc.tile_pool(name="s_oe", bufs=3))
        s_ech = ctx.enter_context(tc.tile_pool(name="s_ech", bufs=2))
        s_zs = ctx.enter_context(tc.tile_pool(name="s_zs", bufs=3))
        s_ex = ctx.enter_context(tc.tile_pool(name="s_ex", bufs=1))
        s_exb = ctx.enter_context(tc.tile_pool(name="s_exb", bufs=1))
        s_sm = ctx.enter_context(tc.tile_pool(name="s_sm", bufs=1))
        s_h = ctx.enter_context(tc.tile_pool(name="s_h", bufs=2))
        s_o = ctx.enter_context(tc.tile_pool(name="s_o", bufs=1))

        def wload(dram, shape, dt=BF16):
            t = wp.tile(list(shape), dt, tag=dram.name, name=dram.name)
            nc.sync.dma_start(t[:], dram[:])
            return t

        ownT_s = wload(ownT, [OWN_D + 1, bc])
        ownW_s = wload(ownW, [OWN_D + 1, D])
        intW_s = wload(intW, [INT_D + 1, D])
        wq_s = wload(wq, [D, D])
        wk_s = wload(wk, [D, D])
        vattm_s = wload(vattm, [D, NI * NI])
        id32_s = wload(id32, [NI, NI])
        ones32_s = wload(ones32, [NI, 1])
        wvp_s = wload(wvp, [D, D])
        h1oe0_s = wload(h1oe0, [D, D]); h1oe1_s = wload(h1oe1, [D, D])
        h1at0_s = wload(h1at0, [D, D]); h1at1_s = wload(h1at1, [D, D])
        h2a0_s = wload(h2a0, [D, D]); h2a1_s = wload(h2a1, [D, D])
        h2b0_s = wload(h2b0, [D, D]); h2b1_s = wload(h2b1, [D, D])
        outlo_s = wload(outlo, [D, OUT_D]); outhi_s = wload(outhi, [D, OUT_D])
        br_s = wload(biasrows, [1, 6 * D])
        ones_s = wload(onesrow, [1, b_tile])

        tile_off = [0]
        for c in cs:
            tile_off.append(tile_off[-1] + c * b_tile)

        # ---------------- per-tile emission ----------------
        def dma_intr(t):
            c = cs[t]
            it = s_intr.tile([INT_D + 1, cmax * b_tile], BF16, tag="intr",
                             name="it")
            nc.sync.dma_start(it[:, 0:c * b_tile],
                              intrT[:, tile_off[t]:tile_off[t + 1]])
            mk = s_mask.tile([NI, b_tile], BF16, tag="mask", name="mk")
            nc.sync.dma_start(mk[:], maskT[t])
            return it, mk

        def emit_T(t, st, pending, pop):
            """T-phase for tile t. st carries tile state; pending/pop
            interleave closures from older tiles."""
            c = cs[t]
            s0 = t * b_tile
            it, mk = st["intr"]
            # own embedding
            poe = p_z.tile([D, b_tile], F32, tag="z", name="poe")
            nc.tensor.matmul(poe[:], ownW_s[:], ownT_s[:, s0:s0 + b_tile])
            oe = s_oe.tile([D, b_tile], BF16, tag="oe", name="oe")
            nc.scalar.activation(oe[:], poe[:], AF.Prelu, alpha=NEG_SLOPE)
            st["oe"] = oe
            ie = s_ie.tile([D, cmax * b_tile], BF16, tag="ie", name="ie")
            st["ie"] = ie
            sct = p_s.tile([NI, b_tile], F32, tag="sct", name="sct")
            st["sct"] = sct

            pairs = [(2 * j, min(2 * j + 1, c - 1)) for j in range((c + 1) // 2)]
            npairs = len(pairs)
            pech = {}
            echch = {}

            def z_lrelu(n):
                pz = p_z.tile([D, b_tile], F32, tag="z", name="pz")
                nc.tensor.matmul(
                    pz[:], intW_s[:],
                    it[:, n * b_tile:(n + 1) * b_tile])
                dst = ie[:, n * b_tile:(n + 1) * b_tile]
                r = _lrelu_route(n)
                if r == "A":
                    nc.scalar.activation(dst, pz[:], AF.Prelu,
                                         alpha=NEG_SLOPE)
                elif r == "D":
                    zs = s_zs.tile([D, b_tile], BF16, tag="zs", name="zs")
                    nc.vector.tensor_scalar_mul(zs[:], pz[:], NEG_SLOPE)
                    nc.vector.tensor_tensor(dst, zs[:], pz[:], op=ALU.max)
                else:  # fallback = D route
                    zs = s_zs.tile([D, b_tile], BF16, tag="zs", name="zs")
                    nc.vector.tensor_scalar_mul(zs[:], pz[:], NEG_SLOPE)
                    nc.vector.tensor_tensor(dst, zs[:], pz[:], op=ALU.max)

            def qk(jp):
                n0, n1 = pairs[jp]
                pe = p_e.tile([D, 2 * b_tile], F32, tag="e", name="pe")
                pech[jp] = pe
                for h, n in enumerate(dict.fromkeys((n0, n1))):
                    half = pe[:, h * b_tile:(h + 1) * b_tile]
                    src = ie[:, n * b_tile:(n + 1) * b_tile]
                    if Q_DVE_EVERY and n % Q_DVE_EVERY == Q_DVE_EVERY - 1:
                        nc.tensor.matmul(half, wk_s[:], src)
                        nc.vector.tensor_tensor(half, half, st["oe"][:],
                                                op=ALU.add)
                    else:
                        nc.tensor.matmul(half, wk_s[:], src,
                                         start=True, stop=False)
                        nc.tensor.matmul(half, wq_s[:], st["oe"][:],
                                         start=False, stop=True)
                w = b_tile if n1 == n0 else 2 * b_tile
                ech = s_ech.tile([D, 2 * b_tile], BF16, tag="ech", name="ech")
                nc.scalar.activation(ech[:, 0:w], pe[:, 0:w], AF.Tanh)
                echch[jp] = ech

            def sc(jp):
                n0, n1 = pairs[jp]
                ech = echch.pop(jp)
                for h, n in enumerate(dict.fromkeys((n0, n1))):
                    nc.tensor.matmul(
                        sct[:], vattm_s[:, n * NI:(n + 1) * NI],
                        ech[:, h * b_tile:(h + 1) * b_tile],
                        start=(n == 0), stop=False, skip_group_check=True)

            for j in range(npairs + 2):
                if j < npairs:
                    for n in dict.fromkeys(pairs[j]):
                        z_lrelu(n)
                if 1 <= j <= npairs:
                    qk(j - 1)
                if 2 <= j <= npairs + 1:
                    sc(j - 2)
                pop()
            # pad-slot mask (exact also when no pads: mask rows are 0)
            nc.tensor.matmul(sct[:], id32_s[0:c, :], mk[0:c, :],
                             start=False, stop=True, skip_group_check=True)

        def steps_A(t, st):
            """Attention phase closures for tile t (run during t+1).
            Critical chain: exp -> dump -> bcast -> muls -> tree -> norm.
            The reciprocal (sum/recip/broadcast) runs in parallel and is
            applied once at the end on the reduced ctx."""
            c = cs[t]
            box = {}

            def s_exp():
                ex = s_ex.tile([NI, b_tile], BF16, tag="ex", name="ex")
                nc.scalar.activation(ex[0:c, :], st["sct"][0:c, :], AF.Exp)
                nc.sync.dma_start(exd[t][0:c * b_tile], ex[0:c, :])
                box["ex"] = ex

            def s_sum():
                ps = p_m.tile([D, b_tile], F32, tag="pm", name="ps")
                nc.tensor.matmul(ps[0:1, :], ones32_s[0:c, 0:1],
                                 box["ex"][0:c, :])
                rs = s_sm.tile([1, b_tile], F32, tag="rs", name="rs")
                nc.vector.reciprocal(rs[:], ps[0:1, :])
                rb = s_sm.tile([1, b_tile], BF16, tag="rb", name="rb")
                nc.vector.tensor_copy(rb[:], rs[:])
                box["rb"] = rb

            def s_rb128():
                rb128 = s_sm.tile([D, b_tile], BF16, tag="rb128",
                                  name="rb128")
                nc.gpsimd.partition_broadcast(rb128[:], box["rb"][0:1, :])
                box["rb128"] = rb128

            def s_bcast(g):
                def f():
                    if "exb" not in box:
                        box["exb"] = s_exb.tile([D, cmax * b_tile], BF16,
                                                tag="exb", name="exb")
                    exb = box["exb"]
                    lo = g * EXB_GROUP
                    hi = min(c, lo + EXB_GROUP)
                    w = (hi - lo) * b_tile
                    src = AP(exd, t * NI * b_tile + lo * b_tile,
                             [[0, D], [1, w]])
                    nc.sync.dma_start(
                        exb[:, lo * b_tile:lo * b_tile + w], src)
                return f

            def s_mul(n0, n1, k):
                def f():
                    w = (n1 - n0) * b_tile
                    eng = (nc.gpsimd if MUL_POOL_EVERY and
                           k % MUL_POOL_EVERY == MUL_POOL_EVERY - 1
                           else nc.vector)
                    eng.tensor_tensor(
                        box["exb"][:, n0 * b_tile:n0 * b_tile + w],
                        box["exb"][:, n0 * b_tile:n0 * b_tile + w],
                        st["ie"][:, n0 * b_tile:n0 * b_tile + w],
                        op=ALU.mult)
                return f

            def s_tree(width, rem, half):
                def f():
                    prod = box["exb"]
                    last = rem == 1
                    pw = int(half * b_tile * TREE_POOL_FRAC) & ~1
                    dw = half * b_tile - pw
                    if last:
                        # final add writes the small ctx tile, freeing exb
                        ctx_t = s_sm.tile([D, b_tile], BF16, tag="ctx",
                                          name="ctx_t", bufs=2)
                        box["ctx"] = ctx_t
                        dst0 = ctx_t[:, 0:dw]
                        dst1 = ctx_t[:, dw:dw + pw]
                    else:
                        dst0 = prod[:, 0:dw]
                        dst1 = prod[:, dw:dw + pw]
                    if dw:
                        nc.vector.tensor_tensor(
                            dst0, prod[:, 0:dw],
                            prod[:, rem * b_tile:rem * b_tile + dw],
                            op=ALU.add)
                    if pw:
                        nc.gpsimd.tensor_tensor(
                            dst1, prod[:, dw:dw + pw],
                            prod[:, rem * b_tile + dw:
                                 rem * b_tile + dw + pw], op=ALU.add)
                return f

            def s_norm():
                nc.vector.tensor_tensor(box["ctx"][:], box["ctx"][:],
                                        box["rb128"][:], op=ALU.mult)

            steps = [(0, s_exp), (1, s_sum), (2, s_rb128)]
            ngroups = (c + EXB_GROUP - 1) // EXB_GROUP
            mul_plan = []
            for g in range(ngroups):
                steps.append((1 + g, s_bcast(g)))
                lo, hi = g * EXB_GROUP, min(c, (g + 1) * EXB_GROUP)
                n = lo
                while n < hi:
                    n2 = min(n + 2, hi)
                    mul_plan.append((n, n2))
                    n = n2
            nm = len(mul_plan)
            for k, (n0, n1) in enumerate(mul_plan):
                steps.append((3 + (7 * k) // max(nm, 1), s_mul(n0, n1, k)))
            w = c
            lev = 0
            while w > 1:
                half = w // 2
                rem = w - half
                steps.append((10 + lev // 2, s_tree(w, rem, half)))
                lev += 1
                w = rem
            steps.append((13, s_norm))
            st["box"] = box
            return steps

        def steps_H(t, st):
            """Head closures for tile t (ctxpre = prod[:, 0:b_tile])."""
            c = cs[t]
            s0 = t * b_tile
            box = st["box"]
            hb = {}

            def bias_mm(ph, k):
                # ph += biasrow_k^T @ onesrow  (only when biases nonzero)
                if with_bias_mms:
                    nc.tensor.matmul(ph, br_s[0:1, k * D:(k + 1) * D],
                                     ones_s[0:1, :],
                                     start=False, stop=True,
                                     skip_group_check=True)

            def mm2(w0, in0, w1, in1, k, cols=D):
                ph = p_m.tile([D, b_tile], F32, tag="pm", name="ph")
                nc.tensor.matmul(ph[0:cols, :], w0[:, 0:cols], in0,
                                 start=True, stop=False,
                                 skip_group_check=True)
                nc.tensor.matmul(ph[0:cols, :], w1[:, 0:cols], in1,
                                 start=False, stop=(not with_bias_mms),
                                 skip_group_check=True)
                bias_mm(ph[0:cols, :], k)
                return ph

            def l_attn():
                ph = p_m.tile([D, b_tile], F32, tag="pm", name="ph")
                nc.tensor.matmul(ph[:], wvp_s[:], box["ctx"][:],
                                 start=True, stop=(not with_bias_mms),
                                 skip_group_check=True)
                bias_mm(ph[:], 0)
                at = s_h.tile([D, b_tile], BF16, tag="attn", name="at")
                nc.scalar.activation(at[:], ph[:], AF.Tanh)
                hb["attn"] = at

            def l_h1(i):
                def f():
                    ph = mm2(h1oe0_s if i == 0 else h1oe1_s, st["oe"][:],
                             h1at0_s if i == 0 else h1at1_s, hb["attn"][:],
                             1 + i)
                    hh = s_h.tile([D, b_tile], BF16, tag=f"h1{i}", name="hh")
                    nc.scalar.activation(hh[:], ph[:], AF.Prelu,
                                         alpha=NEG_SLOPE)
                    hb[f"h1{i}"] = hh
                return f

            def l_h2(i):
                def f():
                    ph = mm2(h2a0_s if i == 0 else h2a1_s, hb["h10"][:],
                             h2b0_s if i == 0 else h2b1_s, hb["h11"][:],
                             3 + i)
                    hh = s_h.tile([D, b_tile], BF16, tag=f"h2{i}", name="hh")
                    nc.scalar.activation(hh[:], ph[:], AF.Prelu,
                                         alpha=NEG_SLOPE)
                    hb[f"h2{i}"] = hh
                return f

            def l_out():
                ph = p_m.tile([D, b_tile], F32, tag="pm", name="ph")
                nc.tensor.matmul(ph[0:OUT_D, :], outlo_s[:], hb["h20"][:],
                                 start=True, stop=False,
                                 skip_group_check=True)
                nc.tensor.matmul(ph[0:OUT_D, :], outhi_s[:], hb["h21"][:],
                                 start=False, stop=(not with_bias_mms),
                                 skip_group_check=True)
                if with_bias_mms:
                    nc.tensor.matmul(ph[0:OUT_D, :],
                                     br_s[0:1, 5 * D:5 * D + OUT_D],
                                     ones_s[0:1, :], start=False, stop=True,
                                     skip_group_check=True)
                osb = s_o.tile([OUT_D, b_tile], F32, tag="o", name="osb")
                nc.vector.tensor_copy(osb[:], ph[0:OUT_D, :])
                nc.sync.dma_start(y[:, s0:s0 + b_tile], osb[:])

            return [(15, l_attn), (16, l_h1(0)), (17, l_h1(1)),
                    (18, l_h2(0)), (19, l_h2(1)), (20, l_out)]

        # ---------------- pipeline (global chunk scheduler) ----------------
        sched = {"G": 0, "pending": []}

        def pop():
            sched["G"] += 1
            g = sched["G"]
            rest, run = [], []
            for due, f in sched["pending"]:
                (run if due <= g else rest).append((due, f))
            sched["pending"] = rest
            for _, f in sorted(run, key=lambda x: x[0]):
                f()

        intr_next = dma_intr(0)
        for t in range(nt):
            st = {"intr": intr_next}
            if t + 1 < nt:
                intr_next = dma_intr(t + 1)
            g0 = sched["G"]
            emit_T(t, st, None, pop)
            for due, f in steps_A(t, st) + steps_H(t, st):
                sched["pending"].append((sched["G"] + due + 1, f))
        for _, f in sorted(sched["pending"], key=lambda x: x[0]):
            f()

    nc.compile()
    return nc


# ---------------- host-side prep ----------------

_CACHED = {}


def prep_inputs(obs, own_W, own_b, int_W, int_b, Wq, Wk, Wv, v_att,
                proj_W, proj_b, h1_W, h1_b, h2_W, h2_b, out_W, out_b):
    obs = np.asarray(obs, np.float32)
    B = obs.shape[0]
    assert B == B_FULL
    f32 = lambda a: np.ascontiguousarray(np.asarray(a, np.float32))
    bf = lambda a: np.ascontiguousarray(
        np.asarray(a, np.float32).astype(BF16_NP))

    own = obs[:, :OWN_D]
    intr = obs[:, OWN_D:].reshape(B, NI, INT_D)
    live = np.abs(intr).sum(2) >= 1e-6          # [B, NI]
    cnt = live.sum(1).astype(np.int64)          # [B]
    assert cnt.min() >= 1, "all-padded sample: compaction unsupported"

    order = np.argsort(cnt, kind="stable")
    nt = NT
    # tile position j on every core uses c*_j = max count in global group j
    cs = tuple(int(cnt[order[(8 * j + 8) * B_TILE - 1]]) for j in range(nt))

    # live-first column permutation per sample
    key = np.argsort(~live, axis=1, kind="stable")      # [B, NI]

    with_bias = any(np.any(np.asarray(b)) for b in
                    (proj_b, h1_b, h2_b, out_b))

    h1_W = np.asarray(h1_W, np.float32)
    h2_W = np.asarray(h2_W, np.float32)
    out_W = np.asarray(out_W, np.float32)
    Wv = np.asarray(Wv, np.float32)
    proj_W = np.asarray(proj_W, np.float32)
    vattm = np.zeros((D, NI, NI), np.float32)
    for n in range(NI):
        vattm[:, n, n] = np.asarray(v_att, np.float32)
    biasrows = np.zeros((1, 6 * D), np.float32)
    biasrows[0, 0:D] = np.asarray(proj_b, np.float32)
    biasrows[0, D:2 * D] = np.asarray(h1_b, np.float32)[:D]
    biasrows[0, 2 * D:3 * D] = np.asarray(h1_b, np.float32)[D:]
    biasrows[0, 3 * D:4 * D] = np.asarray(h2_b, np.float32)[:D]
    biasrows[0, 4 * D:5 * D] = np.asarray(h2_b, np.float32)[D:]
    biasrows[0, 5 * D:5 * D + OUT_D] = np.asarray(out_b, np.float32)

    shared = dict(
        ownW=bf(np.concatenate([np.asarray(own_W, np.float32),
                                np.asarray(own_b, np.float32)[None]], 0)),
        intW=bf(np.concatenate([np.asarray(int_W, np.float32),
                                np.asarray(int_b, np.float32)[None]], 0)),
        wq=bf(Wq), wk=bf(Wk),
        vattm=bf(vattm.reshape(D, NI * NI)),
        id32=bf(np.eye(NI)),
        ones32=bf(np.ones((NI, 1))),
        wvp=bf(Wv @ proj_W),
        h1oe0=bf(h1_W[:D, :D]), h1oe1=bf(h1_W[:D, D:]),
        h1at0=bf(h1_W[D:, :D]), h1at1=bf(h1_W[D:, D:]),
        h2a0=bf(h2_W[:D, :D]), h2a1=bf(h2_W[:D, D:]),
        h2b0=bf(h2_W[D:, :D]), h2b1=bf(h2_W[D:, D:]),
        outlo=bf(out_W[:D]), outhi=bf(out_W[D:]),
        biasrows=bf(biasrows),
        onesrow=bf(np.ones((1, B_TILE))),
    )

    in_maps = []
    perms = []
    for i in range(N_CORES):
        perm_i = np.concatenate(
            [order[(8 * j + i) * B_TILE:(8 * j + i + 1) * B_TILE]
             for j in range(nt)])
        perms.append(perm_i)
        intr_cols = []
        masks = np.zeros((nt, NI, B_TILE), np.float32)
        for j in range(nt):
            c = cs[j]
            idx = perm_i[j * B_TILE:(j + 1) * B_TILE]
            ic = np.take_along_axis(intr[idx], key[idx][:, :c, None],
                                    axis=1)          # [b, c, 7] live-first
            pad = np.arange(c)[None, :] >= cnt[idx][:, None]   # [b, c]
            ic = np.where(pad[:, :, None], 0.0, ic)
            # [8, c, b]: 7 features + ones row
            blk = np.concatenate(
                [ic.transpose(2, 1, 0),
                 np.ones((1, c, B_TILE), np.float32)], 0)
            intr_cols.append(blk.reshape(INT_D + 1, c * B_TILE))
            masks[j, :c, :] = np.where(pad.T, NEG_BIG, 0.0)
        ownT_i = np.concatenate(
            [own[perm_i].T, np.ones((1, BC), np.float32)], 0)
        in_maps.append(dict(
            shared,
            intrT=bf(np.concatenate(intr_cols, 1)),
            ownT=bf(ownT_i),
            maskT=bf(masks),
        ))

    _CACHED["cs"] = cs
    _CACHED["perms"] = perms
    _CACHED["with_bias"] = with_bias
    key_ = (cs, with_bias)
    if _CACHED.get("key") != key_:
        _CACHED["nc"] = build_program(cs, with_bias_mms=with_bias)
        _CACHED["key"] = key_
    return in_maps


def _get_program():
    if "nc" not in _CACHED:
        # default program (uniform worst case) for standalone inspection
        _CACHED["nc"] = build_program(tuple([NI] * NT))
        _CACHED["key"] = (tuple([NI] * NT), False)
    return _CACHED["nc"]


def run_on_device(in_maps, trace=False):
    from concourse.bass_utils import run_bass_kernel_spmd
    nc = _get_program()
    return run_bass_kernel_spmd(nc, in_maps,
                                core_ids=list(range(len(in_maps))),
                                trace=trace)


def postprocess(res):
    B = B_FULL
    out = np.empty((B, OUT_D), np.float32)
    for i, r in enumerate(res.results):
        out[_CACHED["perms"][i]] = np.asarray(r["y"]).T
    return out


def kernel(**inputs):
    in_maps = prep_inputs(**inputs)
    try:
        res = run_on_device(in_maps)
    except Exception:
        import time as _time
        _time.sleep(10)
        res = run_on_device(in_maps)
    return postprocess(res)
